# revision 16
# baseline (speedup 1.0000x reference)
"""Block-sparse 3-layer MLP on 8 Trainium2 NeuronCores.

Reference computation (fp32):
    h1 = relu(x @ (W1*expand(mask1)).T + b1)       x:[B,2048] W1:[4096,2048]
    h2 = relu(h1 @ (W2*expand(mask2)).T + b2)      W2:[4096,4096]
    out = h2 @ Wo.T + bo                           Wo:[1024,4096] -> [B,1024]

Strategy: data-parallel over the batch (B=8192 -> 1024 rows/core), no
collectives. Masks are applied to the weights on the host (free), and all
matmuls run dense on the PE array in bf16 (1 cycle/row; fp32 PSUM
accumulation keeps rel err ~4e-3, threshold is 2e-2).

Per core, three sequential dense phases, PE-streaming-bound:
  L1: 32 m-tiles x 16 k x 2 strips = 1024 MMs, h1 kept resident in SBUF.
      x is spread over all 3 DMA rings so k-tiles arrive as fast as the
      PE consumes them; a burst of junk warmup MMs lifts the HAM clock
      gate to 8/8 before real work lands.
  L2: 32 m-tiles x 32 k x 2 strips = 2048 MMs, h2 resident (bf16); Wo
      panels trickle in during L2 so L3 starts with zero DMA deps.
  L3: trailing dense head: 8 mo x 32 k x 2 strips = 512 MMs; per-mo
      ACT(+bias) -> SBUF -> DMA pipelines, so the tail is one ACT + DMA.
      h1's SBUF is released before L3 to make room for the Wo panels.

MM floor: 3584 x 213.3ns = 765us; measured cadence 216ns/MM.
"""

import sys

sys.path.insert(0, "/opt/trn_rl_repo")

import numpy as np

from concourse import bacc, mybir, tile
from concourse.bass_utils import run_bass_kernel_spmd

F32 = mybir.dt.float32
BF16 = mybir.dt.bfloat16
RELU = mybir.ActivationFunctionType.Relu
IDENT = mybir.ActivationFunctionType.Identity

N_CORES = 8
TILE = 32  # block-sparse tile size of the masks
P = 128  # partitions

MM_DTYPE = "bf16"
WARM_MMS = 12  # junk matmuls to lift the HAM clock gate during DMA wait


def _build(nc, d_in, d_h, d_out, bc, mm_dtype=MM_DTYPE):
    """Emit the per-core kernel. bc = batch columns per core."""
    kt1 = d_in // P  # k-tiles in layer 1 (16)
    mt1 = d_h // P  # m-tiles of h1 (32) == k-tiles of layer 2
    mt2 = d_h // P  # m-tiles of h2 (32) == k-tiles of layer 3
    mot = d_out // P  # m-tiles of out (8)
    sw = min(512, bc)  # psum strip width
    ns = bc // sw  # strips per row of tiles (2)
    kh = 4  # k-tiles per weight panel piece

    sdt = {"bf16": BF16, "f32": F32}[mm_dtype]

    xt_d = nc.dram_tensor("xt", [kt1, P, bc], sdt, kind="ExternalInput")
    w1_d = nc.dram_tensor("w1", [mt1, P, d_in], sdt, kind="ExternalInput")
    b1_d = nc.dram_tensor("b1", [P, mt1], F32, kind="ExternalInput")
    w2_d = nc.dram_tensor("w2", [mt2, P, d_h], sdt, kind="ExternalInput")
    b2_d = nc.dram_tensor("b2", [P, mt2], F32, kind="ExternalInput")
    wo_d = nc.dram_tensor("wo", [mt2, P, d_out], sdt, kind="ExternalInput")
    bo_d = nc.dram_tensor("bo", [P, mot], F32, kind="ExternalInput")
    out_d = nc.dram_tensor("out", [mot, P, bc], F32, kind="ExternalOutput")

    with tile.TileContext(nc) as tc:
        with (
            tc.tile_pool(name="bias", bufs=1) as bias_pool,
            tc.tile_pool(name="h2", bufs=1) as h2_pool,
            tc.tile_pool(name="ps", bufs=4, space="PSUM") as ps_pool,
        ):
            b1_sb = bias_pool.tile([P, mt1], F32, tag="b1")
            b2_sb = bias_pool.tile([P, mt2], F32, tag="b2")
            bo_sb = bias_pool.tile([P, mot], F32, tag="bo")

            h2 = [
                h2_pool.tile([P, bc], sdt, name=f"h2_{i}", tag=f"h2_{i}")
                for i in range(mt2)
            ]
            wo_sb = []

            # h1 + the L1/L2 weight streams live only in this scope
            with (
                tc.tile_pool(name="h1", bufs=1) as h1_pool,
                tc.tile_pool(name="w2pre", bufs=6) as w2pre_pool,
            ):
                h1 = [
                    h1_pool.tile([P, bc], sdt, name=f"h1_{i}", tag=f"h1_{i}")
                    for i in range(mt1)
                ]

                # ---------------- Layer 1 ----------------
                with (
                    tc.tile_pool(name="warm", bufs=1) as warm_pool,
                    tc.tile_pool(name="xtp", bufs=1) as xt_pool,
                    tc.tile_pool(name="w1p", bufs=18) as w1_pool,
                ):
                    # PE warmup: junk matmuls with no DMA deps so the HAM
                    # clock gate reaches 8/8 while x still streams in.
                    wsb = warm_pool.tile([P, P + sw], sdt, tag="warm")
                    nc.gpsimd.memset(wsb[:], 0.0)
                    pswm = ps_pool.tile([P, bc], F32, name="pswm", tag="ps")
                    for _ in range(WARM_MMS):
                        nc.tensor.matmul(
                            pswm[:, :sw], wsb[:, :P], wsb[:, P : P + sw],
                            start=True, stop=True,
                        )

                    # startup DMAs in consumption order, dealt round-robin
                    # over the 3 rings: w1[m0] piece h is needed at k=4h, xt
                    # k-tiles in k order, later w1 head panels after
                    rings = [nc.sync, nc.gpsimd, nc.scalar]
                    xt = [
                        xt_pool.tile([P, bc], sdt, name=f"xt_{kt}", tag=f"xt_{kt}")
                        for kt in range(kt1)
                    ]
                    w1_head = [[None] * (kt1 // kh) for _ in range(3)]
                    need = []
                    for h in range(kt1 // kh):
                        need.append(("w1", 0, h))
                        for k in range(kh):
                            need.append(("xt", h * kh + k))
                    for m in (1, 2):
                        for h in range(kt1 // kh):
                            need.append(("w1", m, h))
                    for idx, item in enumerate(need):
                        ring = rings[idx % 3]
                        if item[0] == "xt":
                            kt = item[1]
                            ring.dma_start(out=xt[kt][:], in_=xt_d[kt])
                        else:
                            _, m, h = item
                            t = w1_pool.tile([P, kh * P], sdt, tag="w1t")
                            ring.dma_start(
                                out=t[:],
                                in_=w1_d[m][:, h * kh * P : (h + 1) * kh * P],
                            )
                            w1_head[m][h] = t

                    def load_w1(mt, ring):
                        w1h = []
                        for h in range(kt1 // kh):
                            t = w1_pool.tile([P, kh * P], sdt, tag="w1t")
                            ring.dma_start(
                                out=t[:],
                                in_=w1_d[mt][:, h * kh * P : (h + 1) * kh * P],
                            )
                            w1h.append(t)
                        return w1h

                    w2pre = []
                    for h in range(6):
                        t = w2pre_pool.tile([P, kh * P], sdt, tag="w2pre")
                        nc.sync.dma_start(
                            out=t[:], in_=w2_d[0][:, h * kh * P : (h + 1) * kh * P]
                        )
                        w2pre.append(t)

                    nc.scalar.dma_start(out=b1_sb[:], in_=b1_d[:])
                    nc.scalar.dma_start(out=b2_sb[:], in_=b2_d[:])
                    nc.scalar.dma_start(out=bo_sb[:], in_=bo_d[:])

                    for mt in range(mt1):
                        w1h = w1_head[mt] if mt < 3 else load_w1(mt, nc.gpsimd)
                        ps = ps_pool.tile([P, bc], F32, name="ps1", tag="ps")
                        for kt in range(kt1):
                            h, r = divmod(kt, kh)
                            for n in range(ns):
                                nc.tensor.matmul(
                                    ps[:, n * sw : (n + 1) * sw],
                                    w1h[h][:, r * P : (r + 1) * P],
                                    xt[kt][:, n * sw : (n + 1) * sw],
                                    start=(kt == 0),
                                    stop=(kt == kt1 - 1),
                                )
                        nc.scalar.activation(
                            h1[mt][:], ps[:], RELU, bias=b1_sb[:, mt : mt + 1]
                        )

                # ---------------- Layers 2+3 ----------------
                # (wop opens only after L1's x/w1 SBUF is released)
                with (
                    tc.tile_pool(name="w2p", bufs=4) as w2_pool,
                    tc.tile_pool(name="wopp", bufs=1) as wo_pool2,
                    tc.tile_pool(name="osb", bufs=1) as o_pool,
                ):
                    def load_w2(mt, pre=None):
                        w2h = list(pre) if pre else []
                        for h in range(len(w2h), mt1 // kh):
                            t = w2_pool.tile([P, kh * P], sdt, tag="w2t")
                            nc.sync.dma_start(
                                out=t[:],
                                in_=w2_d[mt][:, h * kh * P : (h + 1) * kh * P],
                            )
                            w2h.append(t)
                        return w2h

                    for mt in range(mt2):
                        w2h = load_w2(mt, w2pre if mt == 0 else None)
                        ps = ps_pool.tile([P, bc], F32, name="ps2", tag="ps")
                        for kt in range(mt1):
                            h, r = divmod(kt, kh)
                            for n in range(ns):
                                nc.tensor.matmul(
                                    ps[:, n * sw : (n + 1) * sw],
                                    w2h[h][:, r * P : (r + 1) * P],
                                    h1[kt][:, n * sw : (n + 1) * sw],
                                    start=(kt == 0),
                                    stop=(kt == mt1 - 1),
                                )
                        nc.scalar.activation(
                            h2[mt][:], ps[:], RELU, bias=b2_sb[:, mt : mt + 1]
                        )
                        # trickle Wo in during L2 (32 resident panels)
                        t = wo_pool2.tile([P, d_out], sdt, tag=f"wot_{mt}")
                        nc.scalar.dma_start(out=t[:], in_=wo_d[mt])
                        wo_sb.append(t)

                    # ---------- Layer 3 (trailing dense head) ----------
                    # bias-add runs in place on PSUM; DMA goes straight
                    # PSUM -> DRAM (no SBUF bounce). The last m-tile runs
                    # its strips back to back so the final ACT+DMA overlap
                    # the closing matmuls.
                    for mo in range(mot):
                        ps = ps_pool.tile([P, bc], F32, name="ps3", tag="ps")
                        last = mo == mot - 1
                        strips = [range(ns)] if not last else [[n] for n in range(ns)]
                        osb = o_pool.tile([P, bc], F32, tag="osb")
                        for group in strips:
                            for kt in range(mt2):
                                for n in group:
                                    nc.tensor.matmul(
                                        ps[:, n * sw : (n + 1) * sw],
                                        wo_sb[kt][:, mo * P : (mo + 1) * P],
                                        h2[kt][:, n * sw : (n + 1) * sw],
                                        start=(kt == 0),
                                        stop=(kt == mt2 - 1),
                                    )
                            if last:
                                n = group[0]
                                cs = slice(n * sw, (n + 1) * sw)
                                nc.scalar.activation(
                                    osb[:, cs], ps[:, cs], IDENT,
                                    bias=bo_sb[:, mo : mo + 1],
                                )
                                nc.sync.dma_start(
                                    out=out_d[mo][:, cs], in_=osb[:, cs]
                                )
                        if not last:
                            nc.scalar.activation(
                                osb[:], ps[:], IDENT, bias=bo_sb[:, mo : mo + 1]
                            )
                            nc.sync.dma_start(out=out_d[mo], in_=osb[:])

    nc.compile()
    return nc


def _expand_mask(mask, t=TILE):
    return np.repeat(np.repeat(np.asarray(mask, dtype=bool), t, axis=0), t, axis=1)


def _pack_lhsT(w, d_m, d_k):
    """[d_m, d_k] weights -> [d_m/P, P, d_k] panels.

    panel[mt, i, kt*P + j] = w[mt*P + j, kt*P + i], so each [P, P] slice of a
    panel is a ready-to-use lhsT block (partition dim = contraction dim).
    """
    mt, kt = d_m // P, d_k // P
    return np.ascontiguousarray(
        w.reshape(mt, P, kt, P).transpose(0, 3, 2, 1).reshape(mt, P, d_k)
    )


def _pack_out_panels(w, d_m, d_k):
    """[d_m, d_k] weights -> [d_k/P, P, d_m] panels keyed by the k-tile.

    panel[kt, i, mo*P + j] = w[mo*P + j, kt*P + i].
    """
    mt, kt = d_m // P, d_k // P
    return np.ascontiguousarray(
        w.reshape(mt, P, kt, P).transpose(2, 3, 0, 1).reshape(kt, P, d_m)
    )


def _pack_bias(b):
    n = b.shape[0] // P
    return np.ascontiguousarray(b.reshape(n, P).T)


def _run(
    x,
    w1e,
    b1,
    w2e,
    b2,
    wo,
    bo,
    d_in,
    d_h,
    d_out,
    n_cores=N_CORES,
    trace=False,
    mm_dtype=MM_DTYPE,
):
    b = x.shape[0]
    bc = b // n_cores

    nc = bacc.Bacc(
        "TRN2", target_bir_lowering=False, debug=False, num_devices=n_cores
    )
    _build(nc, d_in, d_h, d_out, bc, mm_dtype=mm_dtype)

    np_sdt = mybir.dt.np(BF16) if mm_dtype == "bf16" else np.float32

    def cvt(a):
        return np.ascontiguousarray(a.astype(np_sdt))

    shared = {
        "w1": cvt(_pack_lhsT(w1e, d_h, d_in)),
        "b1": _pack_bias(b1),
        "w2": cvt(_pack_lhsT(w2e, d_h, d_h)),
        "b2": _pack_bias(b2),
        "wo": cvt(_pack_out_panels(wo, d_out, d_h)),
        "bo": _pack_bias(bo),
    }
    in_maps = []
    for c in range(n_cores):
        xc = np.ascontiguousarray(x[c * bc : (c + 1) * bc].T).reshape(
            d_in // P, P, bc
        )
        in_maps.append({"xt": cvt(xc), **shared})

    res = run_bass_kernel_spmd(
        nc, in_maps, core_ids=list(range(n_cores)), trace=trace
    )
    outs = []
    for c in range(n_cores):
        outs.append(res.results[c]["out"].reshape(d_out, bc))
    full = np.concatenate(outs, axis=1)  # [d_out, B]
    return np.ascontiguousarray(full.T), res


def kernel(x, W1, b1, W2, b2, Wo, bo, mask1, mask2):
    x = np.asarray(x, dtype=np.float32)
    w1e = np.asarray(W1, dtype=np.float32) * _expand_mask(mask1)
    w2e = np.asarray(W2, dtype=np.float32) * _expand_mask(mask2)
    out, _ = _run(
        x,
        w1e,
        np.asarray(b1, np.float32),
        w2e,
        np.asarray(b2, np.float32),
        np.asarray(Wo, np.float32),
        np.asarray(bo, np.float32),
        d_in=2048,
        d_h=4096,
        d_out=1024,
    )
    return out


# revision 17
# speedup vs baseline: 1.1858x; 1.1858x over previous
"""Block-sparse 3-layer MLP on 8 Trainium2 NeuronCores.

Reference computation (fp32):
    h1 = relu(x @ (W1*expand(mask1)).T + b1)       x:[B,2048] W1:[4096,2048]
    h2 = relu(h1 @ (W2*expand(mask2)).T + b2)      W2:[4096,4096]
    out = h2 @ Wo.T + bo                           Wo:[1024,4096] -> [B,1024]

Strategy: data-parallel over the batch (B=8192 -> 1024 rows/core), no
collectives. Masks are applied to the weights on the host (free), and all
matmuls run dense on the PE array in bf16 (1 cycle/row; fp32 PSUM
accumulation keeps rel err ~4e-3, threshold is 2e-2).

Per core, three sequential dense phases, PE-streaming-bound:
  L1: 32 m-tiles x 16 k x 2 strips = 1024 MMs, h1 kept resident in SBUF.
      x is spread over all 3 DMA rings so k-tiles arrive as fast as the
      PE consumes them; a burst of junk warmup MMs lifts the HAM clock
      gate to 8/8 before real work lands.
  L2: 32 m-tiles x 32 k x 2 strips = 2048 MMs, h2 resident (bf16); Wo
      panels trickle in during L2 so L3 starts with zero DMA deps.
  L3: trailing dense head: 8 mo x 32 k x 2 strips = 512 MMs; per-mo
      ACT(+bias) -> SBUF -> DMA pipelines, so the tail is one ACT + DMA.
      h1's SBUF is released before L3 to make room for the Wo panels.

MM floor: 3584 x 213.3ns = 765us; measured cadence 216ns/MM.
"""

import sys

sys.path.insert(0, "/opt/trn_rl_repo")

import numpy as np

from concourse import bacc, mybir, tile
from concourse.bass_utils import run_bass_kernel_spmd

F32 = mybir.dt.float32
BF16 = mybir.dt.bfloat16
RELU = mybir.ActivationFunctionType.Relu
IDENT = mybir.ActivationFunctionType.Identity

N_CORES = 8
TILE = 32  # block-sparse tile size of the masks
P = 128  # partitions

MM_DTYPE = "bf16"
WARM_MMS = 16  # junk matmuls to lift the HAM clock gate during DMA wait


def _build(nc, d_in, d_h, d_out, bc, mm_dtype=MM_DTYPE):
    """Emit the per-core kernel. bc = batch columns per core."""
    kt1 = d_in // P  # k-tiles in layer 1 (16)
    mt1 = d_h // P  # m-tiles of h1 (32) == k-tiles of layer 2
    mt2 = d_h // P  # m-tiles of h2 (32) == k-tiles of layer 3
    mot = d_out // P  # m-tiles of out (8)
    sw = min(512, bc)  # psum strip width
    ns = bc // sw  # strips per row of tiles (2)
    kh = 4  # k-tiles per weight panel piece

    sdt = {"bf16": BF16, "f32": F32}[mm_dtype]

    xt_d = nc.dram_tensor("xt", [kt1, P, bc], sdt, kind="ExternalInput")
    w1_d = nc.dram_tensor("w1", [mt1, P, d_in], sdt, kind="ExternalInput")
    b1_d = nc.dram_tensor("b1", [P, mt1], F32, kind="ExternalInput")
    w2_d = nc.dram_tensor("w2", [mt2, P, d_h], sdt, kind="ExternalInput")
    b2_d = nc.dram_tensor("b2", [P, mt2], F32, kind="ExternalInput")
    wo_d = nc.dram_tensor("wo", [mt2, P, d_out], sdt, kind="ExternalInput")
    bo_d = nc.dram_tensor("bo", [P, mot], F32, kind="ExternalInput")
    out_d = nc.dram_tensor("out", [mot, P, bc], F32, kind="ExternalOutput")

    with tile.TileContext(nc) as tc:
        with (
            tc.tile_pool(name="bias", bufs=1) as bias_pool,
            tc.tile_pool(name="h2", bufs=1) as h2_pool,
            tc.tile_pool(name="ps", bufs=4, space="PSUM") as ps_pool,
        ):
            b1_sb = bias_pool.tile([P, mt1], F32, tag="b1")
            b2_sb = bias_pool.tile([P, mt2], F32, tag="b2")
            bo_sb = bias_pool.tile([P, mot], F32, tag="bo")

            h2 = [
                h2_pool.tile([P, bc], sdt, name=f"h2_{i}", tag=f"h2_{i}")
                for i in range(mt2)
            ]
            wo_sb = []

            # h1 + the L1/L2 weight streams live only in this scope
            with (
                tc.tile_pool(name="h1", bufs=1) as h1_pool,
                tc.tile_pool(name="w2pre", bufs=6) as w2pre_pool,
            ):
                h1 = [
                    h1_pool.tile([P, bc], sdt, name=f"h1_{i}", tag=f"h1_{i}")
                    for i in range(mt1)
                ]

                # ---------------- Layer 1 ----------------
                with (
                    tc.tile_pool(name="warm", bufs=1) as warm_pool,
                    tc.tile_pool(name="xtp", bufs=1) as xt_pool,
                    tc.tile_pool(name="w1p", bufs=18) as w1_pool,
                ):
                    # PE warmup: junk matmuls with no DMA deps so the HAM
                    # clock gate reaches 8/8 while x still streams in.
                    wsb = warm_pool.tile([P, P + sw], sdt, tag="warm")
                    nc.gpsimd.memset(wsb[:], 0.0)
                    pswm = ps_pool.tile([P, bc], F32, name="pswm", tag="ps")
                    for _ in range(WARM_MMS):
                        nc.tensor.matmul(
                            pswm[:, :sw], wsb[:, :P], wsb[:, P : P + sw],
                            start=True, stop=True,
                        )

                    # startup DMAs in consumption order, dealt round-robin
                    # over the 3 rings: w1[m0] piece h is needed at k=4h, xt
                    # k-tiles in k order, later w1 head panels after
                    rings = [nc.sync, nc.gpsimd, nc.scalar]
                    xt = [
                        xt_pool.tile([P, bc], sdt, name=f"xt_{kt}", tag=f"xt_{kt}")
                        for kt in range(kt1)
                    ]
                    w1_head = [[None] * (kt1 // kh) for _ in range(3)]
                    need = []
                    for h in range(kt1 // kh):
                        need.append(("w1", 0, h))
                        for k in range(kh):
                            kt = h * kh + k
                            need.append(("xt", kt, 0))
                            need.append(("xt", kt, 1))
                    for m in (1, 2):
                        for h in range(kt1 // kh):
                            need.append(("w1", m, h))
                    for idx, item in enumerate(need):
                        ring = rings[idx % 3]
                        if item[0] == "xt":
                            _, kt, half = item
                            cs = slice(half * sw, (half + 1) * sw)
                            ring.dma_start(out=xt[kt][:, cs], in_=xt_d[kt][:, cs])
                        else:
                            _, m, h = item
                            t = w1_pool.tile([P, kh * P], sdt, tag="w1t")
                            ring.dma_start(
                                out=t[:],
                                in_=w1_d[m][:, h * kh * P : (h + 1) * kh * P],
                            )
                            w1_head[m][h] = t

                    def load_w1(mt, ring):
                        w1h = []
                        for h in range(kt1 // kh):
                            t = w1_pool.tile([P, kh * P], sdt, tag="w1t")
                            ring.dma_start(
                                out=t[:],
                                in_=w1_d[mt][:, h * kh * P : (h + 1) * kh * P],
                            )
                            w1h.append(t)
                        return w1h

                    w2pre = []
                    for h in range(6):
                        t = w2pre_pool.tile([P, kh * P], sdt, tag="w2pre")
                        nc.sync.dma_start(
                            out=t[:], in_=w2_d[0][:, h * kh * P : (h + 1) * kh * P]
                        )
                        w2pre.append(t)

                    nc.scalar.dma_start(out=b1_sb[:], in_=b1_d[:])
                    nc.scalar.dma_start(out=b2_sb[:], in_=b2_d[:])
                    nc.scalar.dma_start(out=bo_sb[:], in_=bo_d[:])

                    for mt in range(mt1):
                        w1h = w1_head[mt] if mt < 3 else load_w1(mt, nc.gpsimd)
                        ps = ps_pool.tile([P, bc], F32, name="ps1", tag="ps")
                        for kt in range(kt1):
                            h, r = divmod(kt, kh)
                            for n in range(ns):
                                nc.tensor.matmul(
                                    ps[:, n * sw : (n + 1) * sw],
                                    w1h[h][:, r * P : (r + 1) * P],
                                    xt[kt][:, n * sw : (n + 1) * sw],
                                    start=(kt == 0),
                                    stop=(kt == kt1 - 1),
                                )
                        nc.scalar.activation(
                            h1[mt][:], ps[:], RELU, bias=b1_sb[:, mt : mt + 1]
                        )

                # ---------------- Layers 2+3 ----------------
                # (wop opens only after L1's x/w1 SBUF is released)
                with (
                    tc.tile_pool(name="w2p", bufs=4) as w2_pool,
                    tc.tile_pool(name="wopp", bufs=1) as wo_pool2,
                    tc.tile_pool(name="osb", bufs=1) as o_pool,
                ):
                    def load_w2(mt, pre=None):
                        w2h = list(pre) if pre else []
                        for h in range(len(w2h), mt1 // kh):
                            t = w2_pool.tile([P, kh * P], sdt, tag="w2t")
                            nc.sync.dma_start(
                                out=t[:],
                                in_=w2_d[mt][:, h * kh * P : (h + 1) * kh * P],
                            )
                            w2h.append(t)
                        return w2h

                    for mt in range(mt2):
                        w2h = load_w2(mt, w2pre if mt == 0 else None)
                        ps = ps_pool.tile([P, bc], F32, name="ps2", tag="ps")
                        for kt in range(mt1):
                            h, r = divmod(kt, kh)
                            for n in range(ns):
                                nc.tensor.matmul(
                                    ps[:, n * sw : (n + 1) * sw],
                                    w2h[h][:, r * P : (r + 1) * P],
                                    h1[kt][:, n * sw : (n + 1) * sw],
                                    start=(kt == 0),
                                    stop=(kt == mt1 - 1),
                                )
                        nc.scalar.activation(
                            h2[mt][:], ps[:], RELU, bias=b2_sb[:, mt : mt + 1]
                        )
                        # trickle Wo in during L2 (32 resident panels)
                        t = wo_pool2.tile([P, d_out], sdt, tag=f"wot_{mt}")
                        nc.scalar.dma_start(out=t[:], in_=wo_d[mt])
                        wo_sb.append(t)

                    # ---------- Layer 3 (trailing dense head) ----------
                    # bias-add runs in place on PSUM; DMA goes straight
                    # PSUM -> DRAM (no SBUF bounce). The last m-tile runs
                    # its strips back to back so the final ACT+DMA overlap
                    # the closing matmuls.
                    for mo in range(mot):
                        ps = ps_pool.tile([P, bc], F32, name="ps3", tag="ps")
                        last = mo == mot - 1
                        strips = [range(ns)] if not last else [[n] for n in range(ns)]
                        osb = o_pool.tile([P, bc], F32, tag="osb")
                        for group in strips:
                            for kt in range(mt2):
                                for n in group:
                                    nc.tensor.matmul(
                                        ps[:, n * sw : (n + 1) * sw],
                                        wo_sb[kt][:, mo * P : (mo + 1) * P],
                                        h2[kt][:, n * sw : (n + 1) * sw],
                                        start=(kt == 0),
                                        stop=(kt == mt2 - 1),
                                    )
                            if last:
                                n = group[0]
                                cs = slice(n * sw, (n + 1) * sw)
                                nc.scalar.activation(
                                    osb[:, cs], ps[:, cs], IDENT,
                                    bias=bo_sb[:, mo : mo + 1],
                                )
                                nc.sync.dma_start(
                                    out=out_d[mo][:, cs], in_=osb[:, cs]
                                )
                        if not last:
                            nc.scalar.activation(
                                osb[:], ps[:], IDENT, bias=bo_sb[:, mo : mo + 1]
                            )
                            nc.sync.dma_start(out=out_d[mo], in_=osb[:])

    nc.compile()
    return nc


def _expand_mask(mask, t=TILE):
    return np.repeat(np.repeat(np.asarray(mask, dtype=bool), t, axis=0), t, axis=1)


def _pack_lhsT(w, d_m, d_k):
    """[d_m, d_k] weights -> [d_m/P, P, d_k] panels.

    panel[mt, i, kt*P + j] = w[mt*P + j, kt*P + i], so each [P, P] slice of a
    panel is a ready-to-use lhsT block (partition dim = contraction dim).
    """
    mt, kt = d_m // P, d_k // P
    return np.ascontiguousarray(
        w.reshape(mt, P, kt, P).transpose(0, 3, 2, 1).reshape(mt, P, d_k)
    )


def _pack_out_panels(w, d_m, d_k):
    """[d_m, d_k] weights -> [d_k/P, P, d_m] panels keyed by the k-tile.

    panel[kt, i, mo*P + j] = w[mo*P + j, kt*P + i].
    """
    mt, kt = d_m // P, d_k // P
    return np.ascontiguousarray(
        w.reshape(mt, P, kt, P).transpose(2, 3, 0, 1).reshape(kt, P, d_m)
    )


def _pack_bias(b):
    n = b.shape[0] // P
    return np.ascontiguousarray(b.reshape(n, P).T)


def _run(
    x,
    w1e,
    b1,
    w2e,
    b2,
    wo,
    bo,
    d_in,
    d_h,
    d_out,
    n_cores=N_CORES,
    trace=False,
    mm_dtype=MM_DTYPE,
):
    b = x.shape[0]
    bc = b // n_cores

    nc = bacc.Bacc(
        "TRN2", target_bir_lowering=False, debug=False, num_devices=n_cores
    )
    _build(nc, d_in, d_h, d_out, bc, mm_dtype=mm_dtype)

    np_sdt = mybir.dt.np(BF16) if mm_dtype == "bf16" else np.float32

    def cvt(a):
        return np.ascontiguousarray(a.astype(np_sdt))

    shared = {
        "w1": cvt(_pack_lhsT(w1e, d_h, d_in)),
        "b1": _pack_bias(b1),
        "w2": cvt(_pack_lhsT(w2e, d_h, d_h)),
        "b2": _pack_bias(b2),
        "wo": cvt(_pack_out_panels(wo, d_out, d_h)),
        "bo": _pack_bias(bo),
    }
    in_maps = []
    for c in range(n_cores):
        xc = np.ascontiguousarray(x[c * bc : (c + 1) * bc].T).reshape(
            d_in // P, P, bc
        )
        in_maps.append({"xt": cvt(xc), **shared})

    res = run_bass_kernel_spmd(
        nc, in_maps, core_ids=list(range(n_cores)), trace=trace
    )
    outs = []
    for c in range(n_cores):
        outs.append(res.results[c]["out"].reshape(d_out, bc))
    full = np.concatenate(outs, axis=1)  # [d_out, B]
    return np.ascontiguousarray(full.T), res


def kernel(x, W1, b1, W2, b2, Wo, bo, mask1, mask2):
    x = np.asarray(x, dtype=np.float32)
    w1e = np.asarray(W1, dtype=np.float32) * _expand_mask(mask1)
    w2e = np.asarray(W2, dtype=np.float32) * _expand_mask(mask2)
    out, _ = _run(
        x,
        w1e,
        np.asarray(b1, np.float32),
        w2e,
        np.asarray(b2, np.float32),
        np.asarray(Wo, np.float32),
        np.asarray(bo, np.float32),
        d_in=2048,
        d_h=4096,
        d_out=1024,
    )
    return out
